# revision 23
# baseline (speedup 1.0000x reference)
"""Trainium2 Bass kernel for a gated bilinear-attention GNN (GAT-with-gate).

Math (per batch b):
    h   = x @ W_w.T + W_b                      [N, D]
    e   = h (A + A^T) h^T  (symmetrized bilinear score, one quadratic form)
    att = softmax(where(adj>0, e, 0), axis=1) * adj
    rv  = h; 3x: az = relu(att @ rv);  c = sigmoid([h, az] @ gate_w.T + gate_b)
               rv = c * h + (1 - c) * az

Device strategy: data-parallel over the batch dim, 2 batches per core on 8
cores.  v5 design notes:

  * All matmul operands bf16 (f32 PSUM accumulate): 1 row/cycle PE rate.
  * adj ships as bf16 (exact 0/1 mask); x, weights bf16; output bf16
    (host casts to f32).  Measured rel err ~4e-3 vs the 2e-2 gate.
  * hST comes from its own matmul with host-folded WST = W_w.T (A+A^T).
  * Masking happens after the exp: ACT exps e from PSUM into the bf16 attT
    slab; one DVE scalar_tensor_tensor per slab multiplies by adjT in
    place AND accumulates the softmax denominator.  attT is then
    normalized per partition (its partition IS the softmax output index),
    so hops never touch 1/denom and hop 0's stationary is h-natural.
  * The gate matmul uses 128-replicated-column stationaries: sigmoid
    pre-activations appear as 128 identical PSUM rows and ACT applies
    Sigmoid once per batch-hop on [128, N].  In TRANSPOSED space that
    replicated c128 tile IS the broadcast coefficient, so the whole hop
    combine is three full-width bf16 DVE tensor_tensors per batch:
        hmaT = hT - azT;  ttT = hmaT * c128;  rvT = ttT + azT
    and ONE xbar dma transpose turns rvT into the next hop's natural-
    layout stationary.  (ACT does one LUT swap for the whole kernel:
    all att exps precede all sigmoids in ACT program order; Relu lives
    in both tables.)
  * Emission interleaves the two batches op-by-op (engine queues are
    strictly in-order).
  * Data DMAs ride the gpsimd software-DGE queue; the sync queue carries
    only the 8 xbar transposes (~1.3us fixed ucode cost each).

Host side only re-lays-out inputs (shard, transpose, bf16 cast, degree
metadata, folded weights).  _fixup_waits post-processes the scheduled
program to satisfy this walrus build's one-sync-wait-per-instruction limit.
"""

import sys
from contextlib import ExitStack

import numpy as np

sys.path.insert(0, "/opt/trn_rl_repo")

import concourse.bass as bass
import concourse.tile as tile
from concourse import mybir
from concourse.bass_utils import run_bass_kernel_spmd
import concourse.bass_utils as _bu

import ml_dtypes



B, N, D = 16, 1024, 128
NCORES = 8
BPC = B // NCORES        # batches per core
NB = N // 128            # 128-row blocks per matrix dim
F32 = mybir.dt.float32
BF16 = mybir.dt.bfloat16
OP = mybir.AluOpType
AF = mybir.ActivationFunctionType


def build_nc():
    nc = bass.Bass("TRN2", target_bir_lowering=False, debug=False,
                   num_devices=NCORES)

    adjT = nc.dram_tensor("adjT", [BPC, N, N], BF16, kind="ExternalInput").ap()
    xT = nc.dram_tensor("xT", [BPC, D, N], BF16, kind="ExternalInput").ap()
    ndegT = nc.dram_tensor("ndegT", [D, BPC * NB], F32, kind="ExternalInput").ap()
    # packed consts: [WwT | WST | gw1c128 | gw2c128]
    cb = nc.dram_tensor("cb", [D, 4 * D], BF16, kind="ExternalInput").ap()
    # packed f32 consts: [Wb | SWb | gb]
    cf = nc.dram_tensor("cf", [D, 3], F32, kind="ExternalInput").ap()
    out = nc.dram_tensor("out", [BPC, D, N], BF16, kind="ExternalOutput").ap()

    with tile.TileContext(nc) as tc, ExitStack() as ctx:
        consts = ctx.enter_context(tc.tile_pool(name="consts", bufs=1))
        pse = ctx.enter_context(tc.tile_pool(name="pse", bufs=2, space="PSUM"))
        psa = ctx.enter_context(tc.tile_pool(name="psa", bufs=2, space="PSUM"))
        psg = ctx.enter_context(tc.tile_pool(name="psg", bufs=1, space="PSUM"))
        adj_pool = ctx.enter_context(tc.tile_pool(name="adj", bufs=6))
        att_pool = ctx.enter_context(tc.tile_pool(name="att", bufs=2))
        work = ctx.enter_context(tc.tile_pool(name="work", bufs=2))
        hop = ctx.enter_context(tc.tile_pool(name="hop", bufs=3))
        rv_pool = ctx.enter_context(tc.tile_pool(name="rv", bufs=4))

        cb_sb = consts.tile([D, 4 * D], BF16, tag="cb")
        nc.gpsimd.dma_start(cb_sb[:, :], cb[:, :])
        cf_sb = consts.tile([D, 3], F32, tag="cf")
        nc.gpsimd.dma_start(cf_sb[:, :], cf[:, :])
        wwT_sb = cb_sb[:, 0:D]
        wst_sb = cb_sb[:, D:2 * D]
        gw1_sb = cb_sb[:, 2 * D:3 * D]
        gw2_sb = cb_sb[:, 3 * D:4 * D]
        wb_sb = cf_sb[:, 0:1]
        swb_sb = cf_sb[:, 1:2]
        gb_sb = cf_sb[:, 2:3]

        # PE warm-up on the packed const tile: keeps the PE p-state ramped
        # during the DMA-bound startup.
        for _ in range(12):
            wps = psg.tile([128, N], F32, tag="psg")
            nc.tensor.matmul(wps[:, 0:128], wwT_sb[:, :], wwT_sb[:, :],
                             start=True, stop=True)

        states = [{} for _ in range(BPC)]

        def phase_prologue():
            for b in range(BPC):
                st = states[b]
                xT_sb = work.tile([D, N], BF16, tag="xT")
                nc.gpsimd.dma_start(xT_sb[:, :], xT[b, :, :])
                st["xT"] = xT_sb
            ndeg_sb = consts.tile([D, BPC * NB], F32, tag="ndeg")
            nc.gpsimd.dma_start(ndeg_sb[:, :], ndegT[:, :])
            for b in range(BPC):
                st = states[b]
                st["ndeg"] = ndeg_sb[:, b * NB:(b + 1) * NB]
                # hT[o,n] = WwT^T x + Wb ; hST[e,n] = WST^T x + SWb
                hT_sb = work.tile([D, N], BF16, tag="hT")
                hST_sb = work.tile([D, N], BF16, tag="hST")
                for dst, wmat, bias in ((hT_sb, wwT_sb, wb_sb),
                                        (hST_sb, wst_sb, swb_sb)):
                    ph = pse.tile([128, N], F32, tag="pse")
                    for ih in range(2):
                        nc.tensor.matmul(ph[:, ih * 512:(ih + 1) * 512], wmat,
                                         st["xT"][:, ih * 512:(ih + 1) * 512],
                                         start=True, stop=True)
                    nc.scalar.activation(dst[:, :], ph[:, :], AF.Identity,
                                         bias=bias, scale=1.0)
                hnat_sb = work.tile([128, N], BF16, tag="hnat")
                nc.sync.dma_start_transpose(
                    hnat_sb[:, :].rearrange("p (nb f) -> p nb f", nb=NB),
                    hT_sb[:, :])
                st.update(hT=hT_sb, hST=hST_sb, hnat=hnat_sb)

        def att_gen(b):
            # attT[k, j] = adj[j, k] exp(e[k, j]) / denom[k]:
            # exp on ACT straight from PSUM, in-place mask + denominator
            # accumulate on DVE, then per-partition normalize (partition IS
            # the softmax output index).
            st = states[b]
            att_sb = att_pool.tile([128, NB * N], BF16, tag="att")
            acc_sb = work.tile([D, NB], F32, tag="acc")
            inv_sb = work.tile([D, NB], F32, tag="inv")
            st.update(att=att_sb, acc=acc_sb, inv=inv_sb)
            for jb in range(NB):
                adj_sb = adj_pool.tile([128, N], BF16, tag="adj")
                nc.gpsimd.dma_start(adj_sb[:, :],
                                    adjT[b, jb * 128:(jb + 1) * 128, :])
                pe = pse.tile([128, N], F32, tag="pse")
                for ih in range(2):
                    nc.tensor.matmul(pe[:, ih * 512:(ih + 1) * 512],
                                     st["hST"][:, jb * 128:(jb + 1) * 128],
                                     st["hT"][:, ih * 512:(ih + 1) * 512],
                                     start=True, stop=True)
                slab = att_sb[:, jb * N:(jb + 1) * N]
                nc.scalar.activation(slab, pe[:, :], AF.Exp)
                nc.vector.scalar_tensor_tensor(
                    slab, slab, 1.0, adj_sb[:, :], OP.mult, OP.mult,
                    accum_out=acc_sb[:, jb:jb + 1])
                if jb in (3, 7):
                    lo = jb - 3
                    half = slice(lo, jb + 1)
                    nc.vector.tensor_tensor(inv_sb[:, half], acc_sb[:, half],
                                            st["ndeg"][:, half], OP.add)
                    nc.vector.reciprocal(inv_sb[:, half], inv_sb[:, half])
                    for nb in range(lo, jb + 1):
                        nc.vector.tensor_scalar_mul(
                            att_sb[:, nb * N:(nb + 1) * N],
                            att_sb[:, nb * N:(nb + 1) * N],
                            inv_sb[:, nb:nb + 1])
                yield

        def hop_gen(b, k):
            last = (k == 2)
            st = states[b]
            rvs = st.get("rvs") or st["hnat"]
            azT_sb = hop.tile([128, N], BF16, tag="azT")
            for ih in range(2):
                sl = slice(ih * 512, (ih + 1) * 512)
                paz = psa.tile([128, 512], F32, tag="psa")
                for jb in range(NB):
                    nc.tensor.matmul(
                        paz[:, :], rvs[:, jb * 128:(jb + 1) * 128],
                        st["att"][:, jb * N + ih * 512:
                                  jb * N + (ih + 1) * 512],
                        start=(jb == 0), stop=(jb == NB - 1))
                nc.scalar.activation(azT_sb[:, sl], paz[:, :], AF.Relu)
                yield
            # gate + sigmoid + transposed-space combine + xbar, pipelined in
            # N/2 halves so the next hop's first stationary blocks arrive a
            # full half earlier.  c128's replicated rows ARE the broadcast
            # coefficient in transposed space.
            pg = psg.tile([128, N], F32, tag="psg")
            c128_sb = hop.tile([128, N], BF16, tag="c128")
            hma_sb = hop.tile([128, N], BF16, tag="hma")
            tt_sb = hop.tile([128, N], BF16, tag="tt")
            rvT_sb = hop.tile([128, N], BF16, tag="rvT")
            rv_nat = None
            if not last:
                rv_nat = rv_pool.tile([128, N], BF16, tag="rvs")
            nc.vector.tensor_tensor(hma_sb[:, :], st["hT"][:, :],
                                    azT_sb[:, :], OP.subtract)
            for ih in range(2):
                sl = slice(ih * 512, (ih + 1) * 512)
                nc.tensor.matmul(pg[:, sl], gw1_sb, st["hT"][:, sl],
                                 start=True, stop=False)
                nc.tensor.matmul(pg[:, sl], gw2_sb, azT_sb[:, sl],
                                 start=False, stop=True)
                nc.scalar.activation(c128_sb[:, sl], pg[:, sl], AF.Sigmoid,
                                     bias=gb_sb, scale=1.0)
                nc.vector.tensor_tensor(tt_sb[:, sl], hma_sb[:, sl],
                                        c128_sb[:, sl], OP.mult)
                nc.vector.tensor_tensor(rvT_sb[:, sl], tt_sb[:, sl],
                                        azT_sb[:, sl], OP.add)
                if last:
                    # output stays transposed ([D, N/2] halves); host
                    # transposes after the gather
                    nc.gpsimd.dma_start(out[b][:, sl], rvT_sb[:, sl])
                else:
                    nc.sync.dma_start_transpose(
                        rv_nat[:, sl].rearrange("p (nb f) -> p nb f", nb=4),
                        rvT_sb[:, sl])
                yield
            if not last:
                st["rvs"] = rv_nat
            yield

        def zip_run(*gens):
            gens = list(gens)
            while gens:
                for g in list(gens):
                    try:
                        next(g)
                    except StopIteration:
                        gens.remove(g)

        # software pipeline: att(b1) overlaps hop0(b0); hop(b1,k) overlaps
        # hop(b0,k+1).  Engine queues are in-order, so the overlap comes
        # from round-robin emission of the two phases' chunks.
        phase_prologue()
        zip_run(att_gen(0))
        # hop0(b0)'s matmul/relu chunks overlap att(b1); its sigmoid waits
        # until all att exps are emitted so ACT swaps LUT tables only once.
        ga, gh = att_gen(1), hop_gen(0, 0)
        next(ga); next(gh)
        next(ga); next(gh)
        for _ in range(6):
            next(ga)
        zip_run(ga, gh)
        zip_run(hop_gen(1, 0), hop_gen(0, 1))
        zip_run(hop_gen(1, 1), hop_gen(0, 2))
        zip_run(hop_gen(1, 2))

        # Spare per-engine nops: relocated by _fixup_waits to carry sync
        # waits that walrus cannot fit on compute-instruction structs.
        nop_insts = []
        for eng in (nc.tensor, nc.vector, nc.scalar, nc.gpsimd, nc.sync):
            for _ in range(128):
                nop_insts.append(eng.nop(nofuse=True).ins)

    _fixup_waits(nc, nop_insts)
    return nc


def _fuse_ldweights(nc):
    """Remove the tile framework's pre-split InstLdweights records, merging
    their sync waits/updates into the following (self-loading) Matmult, so
    walrus --enable-ldw-opt=true can overlap stationary loads."""
    f = nc.m.functions[0]
    for blk in f.blocks:
        insts = blk.instructions
        kept = []
        pending = None
        for inst in insts:
            name = inst.__class__.__name__
            if name == "InstLdweights":
                si = inst.sync_info
                assert pending is None
                pending = (list(si.on_wait or []) if si else [],
                           list(si.on_update or []) if si else [])
                continue
            if pending is not None:
                assert name == "InstMatmult", f"ldw followed by {name}"
                w, u = pending
                si = inst.sync_info
                cw = list(si.on_wait or []) if si else []
                cu = list(si.on_update or []) if si else []
                inst.sync_info = mybir.SyncInfo(on_wait=w + cw,
                                                on_update=cu + u)
                pending = None
            kept.append(inst)
        assert pending is None
        if len(kept) != len(insts):
            insts[:] = kept


_FIXUP_SKIP = {"InstNoOp"}


def _fixup_waits(nc, nop_insts):
    """walrus (enable-ldw-opt=false) rejects compute instructions with more
    than one sync wait (single wait slot in the S3 structs).  Hoist
    all-but-one wait of each such instruction onto spare same-engine nop
    instructions inserted immediately before it in program order."""
    nop_set = set(id(x) for x in nop_insts)
    free_nops = {}
    for x in nop_insts:
        free_nops.setdefault(x.engine, []).append(x)
    f = nc.m.functions[0]
    for blk in f.blocks:
        insts = blk.instructions
        for i in range(len(insts) - 1, -1, -1):
            if id(insts[i]) in nop_set:
                insts.pop(i)
        i = 0
        while i < len(insts):
            inst = insts[i]
            if inst.__class__.__name__ not in _FIXUP_SKIP:
                si = inst.sync_info
                if si is not None and si.on_wait and len(si.on_wait) > 1:
                    waits = list(si.on_wait)
                    extra, keep = waits[:-1], waits[-1:]
                    inst.sync_info = mybir.SyncInfo(
                        on_wait=keep, on_update=list(si.on_update or []))
                    pool = free_nops.get(inst.engine)
                    for kk, w in enumerate(extra):
                        if not pool:
                            raise RuntimeError(
                                f"out of spare nops for {inst.engine}")
                        nop = pool.pop()
                        nop.sync_info = mybir.SyncInfo(on_wait=[w], on_update=[])
                        insts.insert(i + kk, nop)
                    i += len(extra)
            i += 1


_NC_CACHE = None


def _get_nc():
    global _NC_CACHE
    if _NC_CACHE is None:
        _NC_CACHE = build_nc()
    return _NC_CACHE


def _bf16(a):
    return np.ascontiguousarray(
        np.asarray(a, dtype=np.float32).astype(ml_dtypes.bfloat16))


def _prep_in_maps(inputs):
    x = np.ascontiguousarray(np.asarray(inputs["x"], dtype=np.float32))
    adj = np.ascontiguousarray(np.asarray(inputs["adj"], dtype=np.float32))
    W_w = np.asarray(inputs["W_w"], dtype=np.float32)
    W_b = np.asarray(inputs["W_b"], dtype=np.float32)
    A = np.asarray(inputs["A"], dtype=np.float32)
    gate_w = np.asarray(inputs["gate_w"], dtype=np.float32)
    gate_b = np.asarray(inputs["gate_b"], dtype=np.float32)

    S = A + A.T
    cb = np.concatenate([
        W_w.T,
        W_w.T @ S,
        np.broadcast_to(gate_w[0, :D].reshape(D, 1), (D, D)),
        np.broadcast_to(gate_w[0, D:].reshape(D, 1), (D, D)),
    ], axis=1)
    cb = _bf16(cb)
    cf = np.ascontiguousarray(
        np.stack([W_b, S @ W_b, np.full(D, gate_b[0])], axis=1),
        dtype=np.float32)

    in_maps = []
    for c in range(NCORES):
        sl = slice(c * BPC, (c + 1) * BPC)
        adj_c = adj[sl]
        adjT_c = _bf16(adj_c.transpose(0, 2, 1))
        xT_c = _bf16(x[sl].transpose(0, 2, 1))
        ndeg = (N - adj_c.sum(axis=1)).astype(np.float32)          # [BPC, N]
        ndegT = np.ascontiguousarray(
            ndeg.reshape(BPC * NB, 128).T)                         # [128, BPC*NB]
        in_maps.append({
            "adjT": adjT_c, "xT": xT_c, "ndegT": ndegT,
            "cb": cb, "cf": cf,
        })
    return in_maps


def _run(inputs, trace=False, **kwargs):
    nc = _get_nc()
    in_maps = _prep_in_maps(inputs)
    res = run_bass_kernel_spmd(nc, in_maps, core_ids=list(range(NCORES)),
                               trace=trace, **kwargs)
    out = np.concatenate(
        [np.asarray(res.results[c]["out"]).astype(np.float32).transpose(0, 2, 1)
         for c in range(NCORES)], axis=0)
    return out, res


def kernel(**inputs) -> np.ndarray:
    out, _ = _run(inputs, trace=False)
    return out


# revision 24
# speedup vs baseline: 1.0341x; 1.0341x over previous
"""Trainium2 Bass kernel for a gated bilinear-attention GNN (GAT-with-gate).

Math (per batch b):
    h   = x @ W_w.T + W_b                      [N, D]
    e   = h (A + A^T) h^T  (symmetrized bilinear score, one quadratic form)
    att = softmax(where(adj>0, e, 0), axis=1) * adj
    rv  = h; 3x: az = relu(att @ rv);  c = sigmoid([h, az] @ gate_w.T + gate_b)
               rv = c * h + (1 - c) * az

Device strategy: data-parallel over the batch dim, 2 batches per core on 8
cores.  v5 design notes:

  * All matmul operands bf16 (f32 PSUM accumulate): 1 row/cycle PE rate.
  * adj ships as bf16 (exact 0/1 mask); x, weights bf16; output bf16
    (host casts to f32).  Measured rel err ~4e-3 vs the 2e-2 gate.
  * hST comes from its own matmul with host-folded WST = W_w.T (A+A^T).
  * Masking happens after the exp: ACT exps e from PSUM into the bf16 attT
    slab; one DVE scalar_tensor_tensor per slab multiplies by adjT in
    place AND accumulates the softmax denominator.  attT is then
    normalized per partition (its partition IS the softmax output index),
    so hops never touch 1/denom and hop 0's stationary is h-natural.
  * The gate matmul uses 128-replicated-column stationaries: sigmoid
    pre-activations appear as 128 identical PSUM rows and ACT applies
    Sigmoid once per batch-hop on [128, N].  In TRANSPOSED space that
    replicated c128 tile IS the broadcast coefficient, so the whole hop
    combine is three full-width bf16 DVE tensor_tensors per batch:
        hmaT = hT - azT;  ttT = hmaT * c128;  rvT = ttT + azT
    and ONE xbar dma transpose turns rvT into the next hop's natural-
    layout stationary.  (ACT does one LUT swap for the whole kernel:
    all att exps precede all sigmoids in ACT program order; Relu lives
    in both tables.)
  * Emission interleaves the two batches op-by-op (engine queues are
    strictly in-order).
  * Data DMAs ride the gpsimd software-DGE queue; the sync queue carries
    only the 8 xbar transposes (~1.3us fixed ucode cost each).

Host side only re-lays-out inputs (shard, transpose, bf16 cast, degree
metadata, folded weights).  _fixup_waits post-processes the scheduled
program to satisfy this walrus build's one-sync-wait-per-instruction limit.
"""

import sys
from contextlib import ExitStack

import numpy as np

sys.path.insert(0, "/opt/trn_rl_repo")

import concourse.bass as bass
import concourse.tile as tile
from concourse import mybir
from concourse.bass_utils import run_bass_kernel_spmd
import concourse.bass_utils as _bu

import ml_dtypes



B, N, D = 16, 1024, 128
NCORES = 8
BPC = B // NCORES        # batches per core
NB = N // 128            # 128-row blocks per matrix dim
F32 = mybir.dt.float32
BF16 = mybir.dt.bfloat16
OP = mybir.AluOpType
AF = mybir.ActivationFunctionType


def build_nc():
    nc = bass.Bass("TRN2", target_bir_lowering=False, debug=False,
                   num_devices=NCORES)

    adjT = nc.dram_tensor("adjT", [BPC, N, N], BF16, kind="ExternalInput").ap()
    xT = nc.dram_tensor("xT", [BPC, D, N], BF16, kind="ExternalInput").ap()
    ndegT = nc.dram_tensor("ndegT", [D, BPC * NB], F32, kind="ExternalInput").ap()
    # packed consts: [WwT | WST | gw1c128 | gw2c128]
    cb = nc.dram_tensor("cb", [D, 4 * D], BF16, kind="ExternalInput").ap()
    # packed f32 consts: [Wb | SWb | gb]
    cf = nc.dram_tensor("cf", [D, 3], F32, kind="ExternalInput").ap()
    out = nc.dram_tensor("out", [BPC, D, N], BF16, kind="ExternalOutput").ap()

    with tile.TileContext(nc) as tc, ExitStack() as ctx:
        consts = ctx.enter_context(tc.tile_pool(name="consts", bufs=1))
        pse = ctx.enter_context(tc.tile_pool(name="pse", bufs=2, space="PSUM"))
        psa = ctx.enter_context(tc.tile_pool(name="psa", bufs=2, space="PSUM"))
        psg = ctx.enter_context(tc.tile_pool(name="psg", bufs=1, space="PSUM"))
        adj_pool = ctx.enter_context(tc.tile_pool(name="adj", bufs=4))
        att_pool = ctx.enter_context(tc.tile_pool(name="att", bufs=2))
        work = ctx.enter_context(tc.tile_pool(name="work", bufs=2))
        hop = ctx.enter_context(tc.tile_pool(name="hop", bufs=3))
        rv_pool = ctx.enter_context(tc.tile_pool(name="rv", bufs=4))

        cb_sb = consts.tile([D, 4 * D], BF16, tag="cb")
        nc.gpsimd.dma_start(cb_sb[:, :], cb[:, :])
        cf_sb = consts.tile([D, 3], F32, tag="cf")
        nc.gpsimd.dma_start(cf_sb[:, :], cf[:, :])
        wwT_sb = cb_sb[:, 0:D]
        wst_sb = cb_sb[:, D:2 * D]
        gw1_sb = cb_sb[:, 2 * D:3 * D]
        gw2_sb = cb_sb[:, 3 * D:4 * D]
        wb_sb = cf_sb[:, 0:1]
        swb_sb = cf_sb[:, 1:2]
        gb_sb = cf_sb[:, 2:3]

        # PE warm-up on the packed const tile: keeps the PE p-state ramped
        # during the DMA-bound startup.
        for _ in range(12):
            wps = psg.tile([128, N], F32, tag="psg")
            nc.tensor.matmul(wps[:, 0:128], wwT_sb[:, :], wwT_sb[:, :],
                             start=True, stop=True)

        states = [{} for _ in range(BPC)]

        def phase_prologue():
            for b in range(BPC):
                st = states[b]
                xT_sb = work.tile([D, N], BF16, tag="xT")
                nc.gpsimd.dma_start(xT_sb[:, :], xT[b, :, :])
                st["xT"] = xT_sb
            ndeg_sb = consts.tile([D, BPC * NB], F32, tag="ndeg")
            nc.gpsimd.dma_start(ndeg_sb[:, :], ndegT[:, :])
            for b in range(BPC):
                st = states[b]
                st["ndeg"] = ndeg_sb[:, b * NB:(b + 1) * NB]
                # hT[o,n] = WwT^T x + Wb ; hST[e,n] = WST^T x + SWb
                hT_sb = work.tile([D, N], BF16, tag="hT")
                hST_sb = work.tile([D, N], BF16, tag="hST")
                for dst, wmat, bias in ((hT_sb, wwT_sb, wb_sb),
                                        (hST_sb, wst_sb, swb_sb)):
                    ph = pse.tile([128, N], F32, tag="pse")
                    for ih in range(2):
                        nc.tensor.matmul(ph[:, ih * 512:(ih + 1) * 512], wmat,
                                         st["xT"][:, ih * 512:(ih + 1) * 512],
                                         start=True, stop=True)
                    nc.scalar.activation(dst[:, :], ph[:, :], AF.Identity,
                                         bias=bias, scale=1.0)
                hnat_sb = work.tile([128, N], BF16, tag="hnat")
                nc.sync.dma_start_transpose(
                    hnat_sb[:, :].rearrange("p (nb f) -> p nb f", nb=NB),
                    hT_sb[:, :])
                st.update(hT=hT_sb, hST=hST_sb, hnat=hnat_sb)

        def att_gen(b):
            # attT[k, j] = adj[j, k] exp(e[k, j]) / denom[k]:
            # exp on ACT straight from PSUM, in-place mask + denominator
            # accumulate on DVE, then per-partition normalize (partition IS
            # the softmax output index).
            st = states[b]
            att_sb = att_pool.tile([128, NB * N], BF16, tag="att")
            acc_sb = work.tile([D, NB], F32, tag="acc")
            inv_sb = work.tile([D, NB], F32, tag="inv")
            st.update(att=att_sb, acc=acc_sb, inv=inv_sb)
            for jb in range(NB):
                adj_sb = adj_pool.tile([128, N], BF16, tag="adj")
                nc.gpsimd.dma_start(adj_sb[:, :],
                                    adjT[b, jb * 128:(jb + 1) * 128, :])
                pe = pse.tile([128, N], F32, tag="pse")
                for ih in range(2):
                    nc.tensor.matmul(pe[:, ih * 512:(ih + 1) * 512],
                                     st["hST"][:, jb * 128:(jb + 1) * 128],
                                     st["hT"][:, ih * 512:(ih + 1) * 512],
                                     start=True, stop=True)
                slab = att_sb[:, jb * N:(jb + 1) * N]
                nc.scalar.activation(slab, pe[:, :], AF.Exp)
                nc.vector.scalar_tensor_tensor(
                    slab, slab, 1.0, adj_sb[:, :], OP.mult, OP.mult,
                    accum_out=acc_sb[:, jb:jb + 1])
                if jb in (3, 7):
                    lo = jb - 3
                    half = slice(lo, jb + 1)
                    nc.vector.tensor_tensor(inv_sb[:, half], acc_sb[:, half],
                                            st["ndeg"][:, half], OP.add)
                    nc.vector.reciprocal(inv_sb[:, half], inv_sb[:, half])
                    for nb in range(lo, jb + 1):
                        nc.vector.tensor_scalar_mul(
                            att_sb[:, nb * N:(nb + 1) * N],
                            att_sb[:, nb * N:(nb + 1) * N],
                            inv_sb[:, nb:nb + 1])
                yield

        def hop_gen(b, k):
            last = (k == 2)
            st = states[b]
            rvs = st.get("rvs") or st["hnat"]
            azT_sb = hop.tile([128, N], BF16, tag="azT")
            for ih in range(2):
                sl = slice(ih * 512, (ih + 1) * 512)
                paz = psa.tile([128, 512], F32, tag="psa")
                for jb in range(NB):
                    nc.tensor.matmul(
                        paz[:, :], rvs[:, jb * 128:(jb + 1) * 128],
                        st["att"][:, jb * N + ih * 512:
                                  jb * N + (ih + 1) * 512],
                        start=(jb == 0), stop=(jb == NB - 1))
                nc.scalar.activation(azT_sb[:, sl], paz[:, :], AF.Relu)
                yield
            # gate + sigmoid + transposed-space combine + xbar, pipelined in
            # N/2 halves so the next hop's first stationary blocks arrive a
            # full half earlier.  c128's replicated rows ARE the broadcast
            # coefficient in transposed space.
            pg = psg.tile([128, N], F32, tag="psg")
            c128_sb = hop.tile([128, N], BF16, tag="c128")
            hma_sb = hop.tile([128, N], BF16, tag="hma")
            tt_sb = hop.tile([128, N], BF16, tag="tt")
            rvT_sb = hop.tile([128, N], BF16, tag="rvT")
            rv_nat = None
            if not last:
                rv_nat = rv_pool.tile([128, N], BF16, tag="rvs")
            nc.vector.tensor_tensor(hma_sb[:, :], st["hT"][:, :],
                                    azT_sb[:, :], OP.subtract)
            for ih in range(2):
                sl = slice(ih * 512, (ih + 1) * 512)
                nc.tensor.matmul(pg[:, sl], gw1_sb, st["hT"][:, sl],
                                 start=True, stop=False)
                nc.tensor.matmul(pg[:, sl], gw2_sb, azT_sb[:, sl],
                                 start=False, stop=True)
                nc.scalar.activation(c128_sb[:, sl], pg[:, sl], AF.Sigmoid,
                                     bias=gb_sb, scale=1.0)
                nc.vector.tensor_tensor(tt_sb[:, sl], hma_sb[:, sl],
                                        c128_sb[:, sl], OP.mult)
                nc.vector.tensor_tensor(rvT_sb[:, sl], tt_sb[:, sl],
                                        azT_sb[:, sl], OP.add)
                if not last:
                    nc.sync.dma_start_transpose(
                        rv_nat[:, sl].rearrange("p (nb f) -> p nb f", nb=4),
                        rvT_sb[:, sl])
                yield
            if last:
                # output stays transposed ([D, N] per batch); host transposes
                nc.gpsimd.dma_start(out[b], rvT_sb[:, :])
            else:
                st["rvs"] = rv_nat
            yield

        def zip_run(*gens):
            gens = list(gens)
            while gens:
                for g in list(gens):
                    try:
                        next(g)
                    except StopIteration:
                        gens.remove(g)

        # software pipeline: att(b1) overlaps hop0(b0); hop(b1,k) overlaps
        # hop(b0,k+1).  Engine queues are in-order, so the overlap comes
        # from round-robin emission of the two phases' chunks.
        phase_prologue()
        zip_run(att_gen(0))
        # hop0(b0)'s matmul/relu chunks overlap att(b1); its sigmoid waits
        # until all att exps are emitted so ACT swaps LUT tables only once.
        ga, gh = att_gen(1), hop_gen(0, 0)
        next(ga); next(gh)
        next(ga); next(gh)
        for _ in range(6):
            next(ga)
        zip_run(ga, gh)
        zip_run(hop_gen(1, 0), hop_gen(0, 1))
        zip_run(hop_gen(1, 1), hop_gen(0, 2))
        zip_run(hop_gen(1, 2))

        # Spare per-engine nops: relocated by _fixup_waits to carry sync
        # waits that walrus cannot fit on compute-instruction structs.
        nop_insts = []
        for eng in (nc.tensor, nc.vector, nc.scalar, nc.gpsimd, nc.sync):
            for _ in range(128):
                nop_insts.append(eng.nop(nofuse=True).ins)

    _fixup_waits(nc, nop_insts)
    return nc


def _fuse_ldweights(nc):
    """Remove the tile framework's pre-split InstLdweights records, merging
    their sync waits/updates into the following (self-loading) Matmult, so
    walrus --enable-ldw-opt=true can overlap stationary loads."""
    f = nc.m.functions[0]
    for blk in f.blocks:
        insts = blk.instructions
        kept = []
        pending = None
        for inst in insts:
            name = inst.__class__.__name__
            if name == "InstLdweights":
                si = inst.sync_info
                assert pending is None
                pending = (list(si.on_wait or []) if si else [],
                           list(si.on_update or []) if si else [])
                continue
            if pending is not None:
                assert name == "InstMatmult", f"ldw followed by {name}"
                w, u = pending
                si = inst.sync_info
                cw = list(si.on_wait or []) if si else []
                cu = list(si.on_update or []) if si else []
                inst.sync_info = mybir.SyncInfo(on_wait=w + cw,
                                                on_update=cu + u)
                pending = None
            kept.append(inst)
        assert pending is None
        if len(kept) != len(insts):
            insts[:] = kept


_FIXUP_SKIP = {"InstNoOp"}


def _fixup_waits(nc, nop_insts):
    """walrus (enable-ldw-opt=false) rejects compute instructions with more
    than one sync wait (single wait slot in the S3 structs).  Hoist
    all-but-one wait of each such instruction onto spare same-engine nop
    instructions inserted immediately before it in program order."""
    nop_set = set(id(x) for x in nop_insts)
    free_nops = {}
    for x in nop_insts:
        free_nops.setdefault(x.engine, []).append(x)
    f = nc.m.functions[0]
    for blk in f.blocks:
        insts = blk.instructions
        for i in range(len(insts) - 1, -1, -1):
            if id(insts[i]) in nop_set:
                insts.pop(i)
        i = 0
        while i < len(insts):
            inst = insts[i]
            if inst.__class__.__name__ not in _FIXUP_SKIP:
                si = inst.sync_info
                if si is not None and si.on_wait and len(si.on_wait) > 1:
                    waits = list(si.on_wait)
                    extra, keep = waits[:-1], waits[-1:]
                    inst.sync_info = mybir.SyncInfo(
                        on_wait=keep, on_update=list(si.on_update or []))
                    pool = free_nops.get(inst.engine)
                    for kk, w in enumerate(extra):
                        if not pool:
                            raise RuntimeError(
                                f"out of spare nops for {inst.engine}")
                        nop = pool.pop()
                        nop.sync_info = mybir.SyncInfo(on_wait=[w], on_update=[])
                        insts.insert(i + kk, nop)
                    i += len(extra)
            i += 1


_NC_CACHE = None


def _get_nc():
    global _NC_CACHE
    if _NC_CACHE is None:
        _NC_CACHE = build_nc()
    return _NC_CACHE


def _bf16(a):
    return np.ascontiguousarray(
        np.asarray(a, dtype=np.float32).astype(ml_dtypes.bfloat16))


def _prep_in_maps(inputs):
    x = np.ascontiguousarray(np.asarray(inputs["x"], dtype=np.float32))
    adj = np.ascontiguousarray(np.asarray(inputs["adj"], dtype=np.float32))
    W_w = np.asarray(inputs["W_w"], dtype=np.float32)
    W_b = np.asarray(inputs["W_b"], dtype=np.float32)
    A = np.asarray(inputs["A"], dtype=np.float32)
    gate_w = np.asarray(inputs["gate_w"], dtype=np.float32)
    gate_b = np.asarray(inputs["gate_b"], dtype=np.float32)

    S = A + A.T
    cb = np.concatenate([
        W_w.T,
        W_w.T @ S,
        np.broadcast_to(gate_w[0, :D].reshape(D, 1), (D, D)),
        np.broadcast_to(gate_w[0, D:].reshape(D, 1), (D, D)),
    ], axis=1)
    cb = _bf16(cb)
    cf = np.ascontiguousarray(
        np.stack([W_b, S @ W_b, np.full(D, gate_b[0])], axis=1),
        dtype=np.float32)

    in_maps = []
    for c in range(NCORES):
        sl = slice(c * BPC, (c + 1) * BPC)
        adj_c = adj[sl]
        adjT_c = _bf16(adj_c.transpose(0, 2, 1))
        xT_c = _bf16(x[sl].transpose(0, 2, 1))
        ndeg = (N - adj_c.sum(axis=1)).astype(np.float32)          # [BPC, N]
        ndegT = np.ascontiguousarray(
            ndeg.reshape(BPC * NB, 128).T)                         # [128, BPC*NB]
        in_maps.append({
            "adjT": adjT_c, "xT": xT_c, "ndegT": ndegT,
            "cb": cb, "cf": cf,
        })
    return in_maps


def _run(inputs, trace=False, **kwargs):
    nc = _get_nc()
    in_maps = _prep_in_maps(inputs)
    res = run_bass_kernel_spmd(nc, in_maps, core_ids=list(range(NCORES)),
                               trace=trace, **kwargs)
    out = np.concatenate(
        [np.asarray(res.results[c]["out"]).astype(np.float32).transpose(0, 2, 1)
         for c in range(NCORES)], axis=0)
    return out, res


def kernel(**inputs) -> np.ndarray:
    out, _ = _run(inputs, trace=False)
    return out


# revision 25
# speedup vs baseline: 1.0430x; 1.0086x over previous
"""Trainium2 Bass kernel for a gated bilinear-attention GNN (GAT-with-gate).

Math (per batch b):
    h   = x @ W_w.T + W_b                      [N, D]
    e   = h (A + A^T) h^T  (symmetrized bilinear score, one quadratic form)
    att = softmax(where(adj>0, e, 0), axis=1) * adj
    rv  = h; 3x: az = relu(att @ rv);  c = sigmoid([h, az] @ gate_w.T + gate_b)
               rv = c * h + (1 - c) * az

Device strategy: data-parallel over the batch dim, 2 batches per core on 8
cores.  v5 design notes:

  * All matmul operands bf16 (f32 PSUM accumulate): 1 row/cycle PE rate.
  * adj ships as bf16 (exact 0/1 mask); x, weights bf16; output bf16
    (host casts to f32).  Measured rel err ~4e-3 vs the 2e-2 gate.
  * hST comes from its own matmul with host-folded WST = W_w.T (A+A^T).
  * Masking happens after the exp: ACT exps e from PSUM into the bf16 attT
    slab; one DVE scalar_tensor_tensor per slab multiplies by adjT in
    place AND accumulates the softmax denominator.  attT is then
    normalized per partition (its partition IS the softmax output index),
    so hops never touch 1/denom and hop 0's stationary is h-natural.
  * The gate matmul uses 128-replicated-column stationaries: sigmoid
    pre-activations appear as 128 identical PSUM rows and ACT applies
    Sigmoid once per batch-hop on [128, N].  In TRANSPOSED space that
    replicated c128 tile IS the broadcast coefficient, so the whole hop
    combine is three full-width bf16 DVE tensor_tensors per batch:
        hmaT = hT - azT;  ttT = hmaT * c128;  rvT = ttT + azT
    and ONE xbar dma transpose turns rvT into the next hop's natural-
    layout stationary.  (ACT does one LUT swap for the whole kernel:
    all att exps precede all sigmoids in ACT program order; Relu lives
    in both tables.)
  * Emission interleaves the two batches op-by-op (engine queues are
    strictly in-order).
  * Data DMAs ride the gpsimd software-DGE queue; the sync queue carries
    only the 8 xbar transposes (~1.3us fixed ucode cost each).

Host side only re-lays-out inputs (shard, transpose, bf16 cast, degree
metadata, folded weights).  _fixup_waits post-processes the scheduled
program to satisfy this walrus build's one-sync-wait-per-instruction limit.
"""

import sys
from contextlib import ExitStack

import numpy as np

sys.path.insert(0, "/opt/trn_rl_repo")

import concourse.bass as bass
import concourse.tile as tile
from concourse import mybir
from concourse.bass_utils import run_bass_kernel_spmd
import concourse.bass_utils as _bu

import ml_dtypes



B, N, D = 16, 1024, 128
NCORES = 8
BPC = B // NCORES        # batches per core
NB = N // 128            # 128-row blocks per matrix dim
F32 = mybir.dt.float32
BF16 = mybir.dt.bfloat16
OP = mybir.AluOpType
AF = mybir.ActivationFunctionType


def build_nc():
    nc = bass.Bass("TRN2", target_bir_lowering=False, debug=False,
                   num_devices=NCORES)

    adjT = nc.dram_tensor("adjT", [BPC, N, N], BF16, kind="ExternalInput").ap()
    xT = nc.dram_tensor("xT", [BPC, D, N], BF16, kind="ExternalInput").ap()
    ndegT = nc.dram_tensor("ndegT", [D, BPC * NB], F32, kind="ExternalInput").ap()
    # packed consts: [WwT | WST | gw1c128 | gw2c128]
    cb = nc.dram_tensor("cb", [D, 4 * D], BF16, kind="ExternalInput").ap()
    # packed f32 consts: [Wb | SWb | gb]
    cf = nc.dram_tensor("cf", [D, 3], F32, kind="ExternalInput").ap()
    out = nc.dram_tensor("out", [BPC, D, N], BF16, kind="ExternalOutput").ap()

    with tile.TileContext(nc) as tc, ExitStack() as ctx:
        consts = ctx.enter_context(tc.tile_pool(name="consts", bufs=1))
        pse = ctx.enter_context(tc.tile_pool(name="pse", bufs=2, space="PSUM"))
        psa = ctx.enter_context(tc.tile_pool(name="psa", bufs=2, space="PSUM"))
        psg = ctx.enter_context(tc.tile_pool(name="psg", bufs=1, space="PSUM"))
        adj_pool = ctx.enter_context(tc.tile_pool(name="adj", bufs=4))
        att_pool = ctx.enter_context(tc.tile_pool(name="att", bufs=2))
        work = ctx.enter_context(tc.tile_pool(name="work", bufs=2))
        hop = ctx.enter_context(tc.tile_pool(name="hop", bufs=3))
        rv_pool = ctx.enter_context(tc.tile_pool(name="rv", bufs=4))

        cb_sb = consts.tile([D, 4 * D], BF16, tag="cb")
        nc.gpsimd.dma_start(cb_sb[:, :], cb[:, :])
        cf_sb = consts.tile([D, 3], F32, tag="cf")
        nc.gpsimd.dma_start(cf_sb[:, :], cf[:, :])
        wwT_sb = cb_sb[:, 0:D]
        wst_sb = cb_sb[:, D:2 * D]
        gw1_sb = cb_sb[:, 2 * D:3 * D]
        gw2_sb = cb_sb[:, 3 * D:4 * D]
        wb_sb = cf_sb[:, 0:1]
        swb_sb = cf_sb[:, 1:2]
        gb_sb = cf_sb[:, 2:3]

        # PE warm-up on the packed const tile: keeps the PE p-state ramped
        # during the DMA-bound startup.
        for _ in range(12):
            wps = psg.tile([128, N], F32, tag="psg")
            nc.tensor.matmul(wps[:, 0:128], wwT_sb[:, :], wwT_sb[:, :],
                             start=True, stop=True)

        states = [{} for _ in range(BPC)]

        def phase_prologue():
            for b in range(BPC):
                st = states[b]
                xT_sb = work.tile([D, N], BF16, tag="xT")
                nc.gpsimd.dma_start(xT_sb[:, :], xT[b, :, :])
                st["xT"] = xT_sb
            ndeg_sb = consts.tile([D, BPC * NB], F32, tag="ndeg")
            nc.gpsimd.dma_start(ndeg_sb[:, :], ndegT[:, :])
            for b in range(BPC):
                st = states[b]
                st["ndeg"] = ndeg_sb[:, b * NB:(b + 1) * NB]
                # hT[o,n] = WwT^T x + Wb ; hST[e,n] = WST^T x + SWb
                hT_sb = work.tile([D, N], BF16, tag="hT")
                hST_sb = work.tile([D, N], BF16, tag="hST")
                for dst, wmat, bias in ((hT_sb, wwT_sb, wb_sb),
                                        (hST_sb, wst_sb, swb_sb)):
                    ph = pse.tile([128, N], F32, tag="pse")
                    for ih in range(2):
                        nc.tensor.matmul(ph[:, ih * 512:(ih + 1) * 512], wmat,
                                         st["xT"][:, ih * 512:(ih + 1) * 512],
                                         start=True, stop=True)
                    nc.scalar.activation(dst[:, :], ph[:, :], AF.Identity,
                                         bias=bias, scale=1.0)
                hnat_sb = work.tile([128, N], BF16, tag="hnat")
                nc.sync.dma_start_transpose(
                    hnat_sb[:, :].rearrange("p (nb f) -> p nb f", nb=NB),
                    hT_sb[:, :])
                st.update(hT=hT_sb, hST=hST_sb, hnat=hnat_sb)

        def att_gen(b):
            # attT[k, j] = adj[j, k] exp(e[k, j]) / denom[k]:
            # exp on ACT straight from PSUM, in-place mask + denominator
            # accumulate on DVE, then per-partition normalize (partition IS
            # the softmax output index).
            st = states[b]
            att_sb = att_pool.tile([128, NB * N], BF16, tag="att")
            acc_sb = work.tile([D, NB], F32, tag="acc")
            inv_sb = work.tile([D, NB], F32, tag="inv")
            st.update(att=att_sb, acc=acc_sb, inv=inv_sb)
            for jb in range(NB):
                adj_sb = adj_pool.tile([128, N], BF16, tag="adj")
                nc.gpsimd.dma_start(adj_sb[:, :],
                                    adjT[b, jb * 128:(jb + 1) * 128, :])
                pe = pse.tile([128, N], F32, tag="pse")
                for ih in range(2):
                    nc.tensor.matmul(pe[:, ih * 512:(ih + 1) * 512],
                                     st["hST"][:, jb * 128:(jb + 1) * 128],
                                     st["hT"][:, ih * 512:(ih + 1) * 512],
                                     start=True, stop=True)
                slab = att_sb[:, jb * N:(jb + 1) * N]
                nc.scalar.activation(slab, pe[:, :], AF.Exp)
                nc.vector.scalar_tensor_tensor(
                    slab, slab, 1.0, adj_sb[:, :], OP.mult, OP.mult,
                    accum_out=acc_sb[:, jb:jb + 1])
                if jb in (3, 7):
                    lo = jb - 3
                    half = slice(lo, jb + 1)
                    nc.vector.tensor_tensor(inv_sb[:, half], acc_sb[:, half],
                                            st["ndeg"][:, half], OP.add)
                    nc.vector.reciprocal(inv_sb[:, half], inv_sb[:, half])
                    for nb in range(lo, jb + 1):
                        nc.vector.tensor_scalar_mul(
                            att_sb[:, nb * N:(nb + 1) * N],
                            att_sb[:, nb * N:(nb + 1) * N],
                            inv_sb[:, nb:nb + 1])
                yield

        def hop_gen(b, k):
            last = (k == 2)
            st = states[b]
            rvs = st.get("rvs") or st["hnat"]
            azT_sb = hop.tile([128, N], BF16, tag="azT")
            for ih in range(2):
                sl = slice(ih * 512, (ih + 1) * 512)
                paz = psa.tile([128, 512], F32, tag="psa")
                for jb in range(NB):
                    nc.tensor.matmul(
                        paz[:, :], rvs[:, jb * 128:(jb + 1) * 128],
                        st["att"][:, jb * N + ih * 512:
                                  jb * N + (ih + 1) * 512],
                        start=(jb == 0), stop=(jb == NB - 1))
                nc.scalar.activation(azT_sb[:, sl], paz[:, :], AF.Relu)
                yield
            # gate + sigmoid + transposed-space combine + xbar, pipelined in
            # N/2 halves so the next hop's first stationary blocks arrive a
            # full half earlier.  c128's replicated rows ARE the broadcast
            # coefficient in transposed space.
            pg = psg.tile([128, N], F32, tag="psg")
            c128_sb = hop.tile([128, N], BF16, tag="c128")
            hma_sb = hop.tile([128, N], BF16, tag="hma")
            tt_sb = hop.tile([128, N], BF16, tag="tt")
            rvT_sb = hop.tile([128, N], BF16, tag="rvT")
            rv_nat = None
            if not last:
                rv_nat = rv_pool.tile([128, N], BF16, tag="rvs")
            nc.vector.tensor_tensor(hma_sb[:, :], st["hT"][:, :],
                                    azT_sb[:, :], OP.subtract)
            for ih in range(2):
                sl = slice(ih * 512, (ih + 1) * 512)
                nc.tensor.matmul(pg[:, sl], gw1_sb, st["hT"][:, sl],
                                 start=True, stop=False)
                nc.tensor.matmul(pg[:, sl], gw2_sb, azT_sb[:, sl],
                                 start=False, stop=True)
                nc.scalar.activation(c128_sb[:, sl], pg[:, sl], AF.Sigmoid,
                                     bias=gb_sb, scale=1.0)
                nc.vector.tensor_tensor(tt_sb[:, sl], hma_sb[:, sl],
                                        c128_sb[:, sl], OP.mult)
                nc.vector.tensor_tensor(rvT_sb[:, sl], tt_sb[:, sl],
                                        azT_sb[:, sl], OP.add)
                if last:
                    # output stays transposed ([D, N/2] halves, overlapped
                    # with the second half's combine); host transposes
                    nc.gpsimd.dma_start(out[b][:, sl], rvT_sb[:, sl])
                else:
                    nc.sync.dma_start_transpose(
                        rv_nat[:, sl].rearrange("p (nb f) -> p nb f", nb=4),
                        rvT_sb[:, sl])
                yield
            if not last:
                st["rvs"] = rv_nat
            yield

        def zip_run(*gens):
            gens = list(gens)
            while gens:
                for g in list(gens):
                    try:
                        next(g)
                    except StopIteration:
                        gens.remove(g)

        # software pipeline: att(b1) overlaps hop0(b0); hop(b1,k) overlaps
        # hop(b0,k+1).  Engine queues are in-order, so the overlap comes
        # from round-robin emission of the two phases' chunks.
        phase_prologue()
        zip_run(att_gen(0))
        # hop0(b0)'s matmul/relu chunks overlap att(b1); its sigmoid waits
        # until all att exps are emitted so ACT swaps LUT tables only once.
        ga, gh = att_gen(1), hop_gen(0, 0)
        next(ga); next(gh)
        next(ga); next(gh)
        for _ in range(6):
            next(ga)
        zip_run(ga, gh)
        zip_run(hop_gen(1, 0), hop_gen(0, 1))
        zip_run(hop_gen(1, 1), hop_gen(0, 2))
        zip_run(hop_gen(1, 2))

        # Spare per-engine nops: relocated by _fixup_waits to carry sync
        # waits that walrus cannot fit on compute-instruction structs.
        nop_insts = []
        for eng in (nc.tensor, nc.vector, nc.scalar, nc.gpsimd, nc.sync):
            for _ in range(128):
                nop_insts.append(eng.nop(nofuse=True).ins)

    _fixup_waits(nc, nop_insts)
    return nc


def _fuse_ldweights(nc):
    """Remove the tile framework's pre-split InstLdweights records, merging
    their sync waits/updates into the following (self-loading) Matmult, so
    walrus --enable-ldw-opt=true can overlap stationary loads."""
    f = nc.m.functions[0]
    for blk in f.blocks:
        insts = blk.instructions
        kept = []
        pending = None
        for inst in insts:
            name = inst.__class__.__name__
            if name == "InstLdweights":
                si = inst.sync_info
                assert pending is None
                pending = (list(si.on_wait or []) if si else [],
                           list(si.on_update or []) if si else [])
                continue
            if pending is not None:
                assert name == "InstMatmult", f"ldw followed by {name}"
                w, u = pending
                si = inst.sync_info
                cw = list(si.on_wait or []) if si else []
                cu = list(si.on_update or []) if si else []
                inst.sync_info = mybir.SyncInfo(on_wait=w + cw,
                                                on_update=cu + u)
                pending = None
            kept.append(inst)
        assert pending is None
        if len(kept) != len(insts):
            insts[:] = kept


_FIXUP_SKIP = {"InstNoOp"}


def _fixup_waits(nc, nop_insts):
    """walrus (enable-ldw-opt=false) rejects compute instructions with more
    than one sync wait (single wait slot in the S3 structs).  Hoist
    all-but-one wait of each such instruction onto spare same-engine nop
    instructions inserted immediately before it in program order."""
    nop_set = set(id(x) for x in nop_insts)
    free_nops = {}
    for x in nop_insts:
        free_nops.setdefault(x.engine, []).append(x)
    f = nc.m.functions[0]
    for blk in f.blocks:
        insts = blk.instructions
        for i in range(len(insts) - 1, -1, -1):
            if id(insts[i]) in nop_set:
                insts.pop(i)
        i = 0
        while i < len(insts):
            inst = insts[i]
            if inst.__class__.__name__ not in _FIXUP_SKIP:
                si = inst.sync_info
                if si is not None and si.on_wait and len(si.on_wait) > 1:
                    waits = list(si.on_wait)
                    extra, keep = waits[:-1], waits[-1:]
                    inst.sync_info = mybir.SyncInfo(
                        on_wait=keep, on_update=list(si.on_update or []))
                    pool = free_nops.get(inst.engine)
                    for kk, w in enumerate(extra):
                        if not pool:
                            raise RuntimeError(
                                f"out of spare nops for {inst.engine}")
                        nop = pool.pop()
                        nop.sync_info = mybir.SyncInfo(on_wait=[w], on_update=[])
                        insts.insert(i + kk, nop)
                    i += len(extra)
            i += 1


_NC_CACHE = None


def _get_nc():
    global _NC_CACHE
    if _NC_CACHE is None:
        _NC_CACHE = build_nc()
    return _NC_CACHE


def _bf16(a):
    return np.ascontiguousarray(
        np.asarray(a, dtype=np.float32).astype(ml_dtypes.bfloat16))


def _prep_in_maps(inputs):
    x = np.ascontiguousarray(np.asarray(inputs["x"], dtype=np.float32))
    adj = np.ascontiguousarray(np.asarray(inputs["adj"], dtype=np.float32))
    W_w = np.asarray(inputs["W_w"], dtype=np.float32)
    W_b = np.asarray(inputs["W_b"], dtype=np.float32)
    A = np.asarray(inputs["A"], dtype=np.float32)
    gate_w = np.asarray(inputs["gate_w"], dtype=np.float32)
    gate_b = np.asarray(inputs["gate_b"], dtype=np.float32)

    S = A + A.T
    cb = np.concatenate([
        W_w.T,
        W_w.T @ S,
        np.broadcast_to(gate_w[0, :D].reshape(D, 1), (D, D)),
        np.broadcast_to(gate_w[0, D:].reshape(D, 1), (D, D)),
    ], axis=1)
    cb = _bf16(cb)
    cf = np.ascontiguousarray(
        np.stack([W_b, S @ W_b, np.full(D, gate_b[0])], axis=1),
        dtype=np.float32)

    in_maps = []
    for c in range(NCORES):
        sl = slice(c * BPC, (c + 1) * BPC)
        adj_c = adj[sl]
        adjT_c = _bf16(adj_c.transpose(0, 2, 1))
        xT_c = _bf16(x[sl].transpose(0, 2, 1))
        ndeg = (N - adj_c.sum(axis=1)).astype(np.float32)          # [BPC, N]
        ndegT = np.ascontiguousarray(
            ndeg.reshape(BPC * NB, 128).T)                         # [128, BPC*NB]
        in_maps.append({
            "adjT": adjT_c, "xT": xT_c, "ndegT": ndegT,
            "cb": cb, "cf": cf,
        })
    return in_maps


def _run(inputs, trace=False, **kwargs):
    nc = _get_nc()
    in_maps = _prep_in_maps(inputs)
    res = run_bass_kernel_spmd(nc, in_maps, core_ids=list(range(NCORES)),
                               trace=trace, **kwargs)
    out = np.concatenate(
        [np.asarray(res.results[c]["out"]).astype(np.float32).transpose(0, 2, 1)
         for c in range(NCORES)], axis=0)
    return out, res


def kernel(**inputs) -> np.ndarray:
    out, _ = _run(inputs, trace=False)
    return out
